# revision 14
# baseline (speedup 1.0000x reference)
"""Single-head dot-product attention on 8 NeuronCores (Trainium2, Bass/Tile).

Problem (per batch element b, data-parallel over the batch of 8):
    q = x @ Wq; k = x @ Wk; v = x @ Wv          x: [2048, 768], W*: [768, 768]
    out = softmax(q @ k.T / sqrt(768)) @ v

Kernel formulation (per core):
  - Fuse the two projection matmuls of the similarity:  q @ k.T = x (Wq Wk^T) x^T,
    so only one projected tensor t = x @ Wqk is needed instead of q and k.
  - Everything is computed in the "transposed" orientation so the softmax
    contraction lands on PE-friendly layouts with zero attention-matrix
    transposes:
        scoresT[k, q] = sum_d xT[d, k] * tT[d, q]           (k on partitions)
        expT = exp(scoresT / sqrt(768))                     (no max-subtraction:
                                                             |scores| <= ~7)
        out_ext[q, 0:769] = sum_k expT[k, q] * [v | 1][k]   (ones column gives the
                                                             softmax denominator)
        out = out_ext[:, :768] * (1 / out_ext[:, 768])
  - Matmul inputs in bf16 (fp32 PSUM accumulation), fp32 everywhere else.
  - Engine balance: PE does matmuls/transposes, ACT does the f32->bf16 input
    casts and the exp, DVE does PSUM->SBUF copies and the final normalization,
    GPSIMD does the ones-column memsets.
"""

import numpy as np

P = 128
S = 2048  # sequence length per core
D = 768   # d_model == q/k/v size
SB = S // P   # 16 s-blocks
DB = D // P   # 6 d-blocks
QSB = 256     # q-superblock (PSUM-bank limited)
NQSB = S // QSB
SCALE = 1.0 / float(np.sqrt(768.0))
N_CORES = 8

_CACHE = {}


def _build_program(reps=1):
    from contextlib import ExitStack

    import concourse.bacc as bacc
    import concourse.mybir as mybir
    import concourse.tile as tile
    from concourse.masks import make_identity

    f32 = mybir.dt.float32
    bf16 = mybir.dt.bfloat16
    EXP = mybir.ActivationFunctionType.Exp

    nc = bacc.Bacc("TRN2", target_bir_lowering=False, debug=False,
                   num_devices=N_CORES)
    x_dram = nc.dram_tensor("x", [S, D], f32, kind="ExternalInput")
    wq_dram = nc.dram_tensor("wq", [D, D], f32, kind="ExternalInput")
    wk_dram = nc.dram_tensor("wk", [D, D], f32, kind="ExternalInput")
    wv_dram = nc.dram_tensor("wv", [D, D], f32, kind="ExternalInput")
    y_dram = nc.dram_tensor("y", [S, D], f32, kind="ExternalOutput")

    with tile.TileContext(nc) as tc:
      for _rep in range(reps):
        top = ExitStack()
        top.__enter__()
        persist = top.enter_context(tc.tile_pool(name="persist", bufs=1))

        ident = persist.tile([P, P], bf16, tag="ident")
        make_identity(nc, ident)

        # Persistent bf16 operands for the attention stage. xT is one 3D
        # tile [P, DB, S] so each s-block's 6 transposed panels land with a
        # single strided DVE copy.
        xT_all = persist.tile([P, DB, S], bf16, tag="xT", name="xT_all")
        xT = [xT_all[:, i, :] for i in range(DB)]
        tT = [persist.tile([P, S], bf16, tag=f"tT{i}", name=f"tT{i}")
              for i in range(DB)]
        vE = [persist.tile([P, 776], bf16, tag=f"vE{i}", name=f"vE{i}")
              for i in range(SB)]

        # ---------------- Phase 1: weights prep + x prep + projections -----
        with ExitStack() as st:
            wstage = st.enter_context(tc.tile_pool(name="wstage", bufs=3))
            wkeep = st.enter_context(tc.tile_pool(name="wkeep", bufs=1))
            xfp = st.enter_context(tc.tile_pool(name="xfp", bufs=8))
            xbp = st.enter_context(tc.tile_pool(name="xbp", bufs=3))
            tp = st.enter_context(
                tc.tile_pool(name="tp", bufs=4, space="PSUM"))
            pa = st.enter_context(
                tc.tile_pool(name="pa", bufs=2, space="PSUM"))
            pb = st.enter_context(
                tc.tile_pool(name="pb", bufs=2, space="PSUM"))

            # Weights first (the first PE work depends on them), then x.
            # Wq/Wk stay f32 (PE transposes them directly; the PSUM->SBUF
            # copy casts to bf16), Wv is cast on ACT for the vE matmuls.
            wqf = [wkeep.tile([P, D], f32, tag=f"wqf{i}", name=f"wqf{i}")
                   for i in range(DB)]
            wkf = [wkeep.tile([P, D], f32, tag=f"wkf{i}", name=f"wkf{i}")
                   for i in range(DB)]
            wv_bf = [wkeep.tile([P, D], bf16, tag=f"wv{i}", name=f"wv{i}")
                     for i in range(DB)]
            # DMA order: wq+wk first (they gate the first PE work), then the
            # first 6 x tiles, then wv, then the rest of x — keeps the
            # startup-critical transfers off shared queues.
            for db in range(DB):
                nc.sync.dma_start(wqf[db], wq_dram[db * P:(db + 1) * P, :])
                nc.sync.dma_start(wkf[db], wk_dram[db * P:(db + 1) * P, :])

            xf = [xfp.tile([P, D], f32, tag="xf", name=f"xf{i}")
                  for i in range(SB)]
            for sb in range(6):
                nc.sync.dma_start(xf[sb], x_dram[sb * P:(sb + 1) * P, :])

            for db in range(DB):
                wf = wstage.tile([P, D], f32, tag="wstage", name="wf")
                nc.sync.dma_start(wf, wv_dram[db * P:(db + 1) * P, :])
                nc.scalar.copy(wv_bf[db], wf)

            for sb in range(6, SB):
                nc.sync.dma_start(xf[sb], x_dram[sb * P:(sb + 1) * P, :])

            # WqT/WkT via f32 PE transpose, db-major: each group needs only
            # ONE weight DMA, so the PE starts as soon as the first weight
            # row lands and then tracks DMA arrivals. 3D layout
            # [P(e), eb, D(d)] keeps the batched strided copies.
            ident_f = wkeep.tile([P, P], f32, tag="ident_f")
            make_identity(nc, ident_f)
            wqT_all = wkeep.tile([P, DB, D], bf16, tag="wqT", name="wqT_all")
            wkT_all = wkeep.tile([P, DB, D], bf16, tag="wkT", name="wkT_all")
            for db in range(DB):
                for src, dstT in ((wqf, wqT_all), (wkf, wkT_all)):
                    pta = pa.tile([P, 512], f32, tag="qk_a", name="pta")
                    ptb = pb.tile([P, 256], f32, tag="qk_b", name="ptb")
                    for eb in range(DB):
                        dst = (pta[:, eb * P:(eb + 1) * P] if eb < 4
                               else ptb[:, (eb - 4) * P:(eb - 3) * P])
                        nc.tensor.transpose(
                            dst, src[db][:, eb * P:(eb + 1) * P], ident_f)
                    nc.vector.tensor_copy(
                        dstT[:, 0:4, db * P:(db + 1) * P],
                        pta.rearrange("p (a b) -> p a b", a=4))
                    nc.vector.tensor_copy(
                        dstT[:, 4:6, db * P:(db + 1) * P],
                        ptb.rearrange("p (a b) -> p a b", a=2))

            # x casts + transposes for the first 6 s-blocks (fills PE while
            # the Wqk inputs finish).
            def x_prep(sb):
                xb = xbp.tile([P, D], bf16, tag="xb", name="xb")
                nc.scalar.copy(xb, xf[sb])
                pt = tp.tile([P, D], bf16, tag="pt", name="pt")
                for db in range(DB):
                    nc.tensor.transpose(
                        pt[:, db * P:(db + 1) * P],
                        xb[:, db * P:(db + 1) * P], ident)
                nc.vector.tensor_copy(
                    xT_all[:, :, sb * P:(sb + 1) * P],
                    pt.rearrange("p (a b) -> p a b", a=DB))

            # v[s, e] = sum_d x[s, d] Wv[d, e]; vE = [v | 1]
            def v_proj(sb):
                pv_a = pa.tile([P, 512], f32, tag="qk_a", name="pv_a")
                pv_b = pb.tile([P, 256], f32, tag="qk_b", name="pv_b")
                for db in range(DB):
                    lhs = xT[db][:, sb * P:(sb + 1) * P]
                    nc.tensor.matmul(pv_a, lhs, wv_bf[db][:, 0:512],
                                     start=(db == 0), stop=(db == DB - 1))
                    nc.tensor.matmul(pv_b, lhs, wv_bf[db][:, 512:768],
                                     start=(db == 0), stop=(db == DB - 1))
                nc.vector.tensor_copy(vE[sb][:, 0:512], pv_a)
                nc.vector.tensor_copy(vE[sb][:, 512:768], pv_b)
                nc.gpsimd.memset(vE[sb][:, 768:769], 1.0)

            # tT[d2, s] = sum_d1 Wqk[d1, d2] xT[d1, s] for 512-chunk g
            def t_proj(g):
                for d2 in range(DB):
                    pj = pa.tile([P, 512], f32, tag="qk_a", name="pj")
                    for db in range(DB):
                        nc.tensor.matmul(
                            pj, wqk[db][:, d2 * P:(d2 + 1) * P],
                            xT[db][:, g * 512:(g + 1) * 512],
                            start=(db == 0), stop=(db == DB - 1))
                    nc.vector.tensor_copy(
                        tT[d2][:, g * 512:(g + 1) * 512], pj)

            for sb in range(6):
                x_prep(sb)

            wqk = [wkeep.tile([P, D], bf16, tag=f"wqk{i}", name=f"wqk{i}")
                   for i in range(DB)]
            for d1 in range(DB):
                qk_a = pa.tile([P, 512], f32, tag="qk_a", name="qk_a")
                qk_b = pb.tile([P, 256], f32, tag="qk_b", name="qk_b")
                for eb in range(DB):
                    lhs = wqT_all[:, eb, d1 * P:(d1 + 1) * P]
                    nc.tensor.matmul(qk_a, lhs, wkT_all[:, eb, 0:512],
                                     start=(eb == 0), stop=(eb == DB - 1))
                    nc.tensor.matmul(qk_b, lhs, wkT_all[:, eb, 512:768],
                                     start=(eb == 0), stop=(eb == DB - 1))
                nc.vector.tensor_copy(wqk[d1][:, 0:512], qk_a)
                nc.vector.tensor_copy(wqk[d1][:, 512:768], qk_b)

            t_proj(0)
            for sb in range(6):
                v_proj(sb)
            for sb in range(6, SB):
                x_prep(sb)
                v_proj(sb)
                if sb == 7:
                    t_proj(1)
                elif sb == 11:
                    t_proj(2)
            t_proj(3)

        # ---------------- Attention stage ---------------------------------
        with ExitStack() as st:
            sc_pool = st.enter_context(
                tc.tile_pool(name="sc", bufs=3, space="PSUM"))
            oa_pool = st.enter_context(
                tc.tile_pool(name="oa", bufs=2, space="PSUM"))
            ob_pool = st.enter_context(
                tc.tile_pool(name="ob", bufs=2, space="PSUM"))
            ex_pool = st.enter_context(tc.tile_pool(name="ex", bufs=3))
            yout = st.enter_context(tc.tile_pool(name="yout", bufs=4))

            for qsb in range(NQSB):
                q0 = qsb * QSB
                oa = [oa_pool.tile([P, 512], f32, tag="oa", name=f"oa{qi}")
                      for qi in range(QSB // P)]
                ob = [ob_pool.tile([P, 257], f32, tag="ob", name=f"ob{qi}")
                      for qi in range(QSB // P)]

                def out_mms(ki, ex):
                    for qi in range(QSB // P):
                        lhs = ex[:, qi * P:(qi + 1) * P]
                        nc.tensor.matmul(oa[qi], lhs, vE[ki][:, 0:512],
                                         start=(ki == 0), stop=(ki == SB - 1))
                        nc.tensor.matmul(ob[qi], lhs, vE[ki][:, 512:769],
                                         start=(ki == 0), stop=(ki == SB - 1))

                prev = None
                for ki in range(SB):
                    sc = sc_pool.tile([P, QSB], f32, tag="sc", name="sc")
                    for db in range(DB):
                        nc.tensor.matmul(
                            sc, xT[db][:, ki * P:(ki + 1) * P],
                            tT[db][:, q0:q0 + QSB],
                            start=(db == 0), stop=(db == DB - 1))
                    ex = ex_pool.tile([P, QSB], bf16, tag="ex", name="ex")
                    nc.scalar.activation(ex, sc, EXP, scale=SCALE)
                    if prev is not None:
                        out_mms(*prev)
                    prev = (ki, ex)
                out_mms(*prev)

                for qi in range(QSB // P):
                    den = yout.tile([P, 1], f32, tag="den", name="den")
                    nc.vector.reciprocal(den, ob[qi][:, 256:257])
                    yt = yout.tile([P, D], f32, tag="yt", name="yt")
                    nc.vector.tensor_scalar_mul(yt[:, 0:512], oa[qi], den)
                    nc.vector.tensor_scalar_mul(
                        yt[:, 512:768], ob[qi][:, 0:256], den)
                    r0 = q0 + qi * P
                    nc.sync.dma_start(y_dram[r0:r0 + P, :], yt)

        top.__exit__(None, None, None)

    nc.compile()
    return nc


def _get_program():
    if "nc" not in _CACHE:
        _CACHE["nc"] = _build_program()
    return _CACHE["nc"]


def _get_runner():
    """Build the program once and wrap it in a cached sharded jit callable."""
    if "runner" in _CACHE:
        return _CACHE["runner"]

    import jax
    from jax.experimental.shard_map import shard_map
    from jax.sharding import Mesh, PartitionSpec

    import concourse.mybir as mybir
    from concourse.bass2jax import (
        _bass_exec_p,
        install_neuronx_cc_hook,
        partition_id_tensor,
    )

    nc = _get_program()
    install_neuronx_cc_hook()

    partition_name = (nc.partition_id_tensor.name
                      if nc.partition_id_tensor else None)
    in_names, out_names, out_avals, zero_shapes = [], [], [], []
    for alloc in nc.m.functions[0].allocations:
        if not isinstance(alloc, mybir.MemoryLocationSet):
            continue
        name = alloc.memorylocations[0].name
        if alloc.kind == "ExternalInput":
            if name != partition_name:
                in_names.append(name)
        elif alloc.kind == "ExternalOutput":
            out_names.append(name)
            shape = tuple(alloc.tensor_shape)
            dtype = mybir.dt.np(alloc.dtype)
            out_avals.append(jax.core.ShapedArray(shape, dtype))
            zero_shapes.append((shape, dtype))
    n_params = len(in_names)
    all_names = list(in_names) + list(out_names)
    if partition_name is not None:
        all_names.append(partition_name)

    def _body(*args):
        operands = list(args)
        if partition_name is not None:
            operands.append(partition_id_tensor())
        outs = _bass_exec_p.bind(
            *operands,
            out_avals=tuple(out_avals),
            in_names=tuple(all_names),
            out_names=tuple(out_names),
            lowering_input_output_aliases=(),
            sim_require_finite=True,
            sim_require_nnan=True,
            nc=nc,
        )
        return tuple(outs)

    devices = jax.devices()[:N_CORES]
    mesh = Mesh(np.asarray(devices), ("core",))
    n_outs = len(out_names)
    sharded = jax.jit(
        shard_map(_body, mesh=mesh,
                  in_specs=(PartitionSpec("core"),) * (n_params + n_outs),
                  out_specs=(PartitionSpec("core"),) * n_outs,
                  check_rep=False),
        donate_argnums=tuple(range(n_params, n_params + n_outs)),
        keep_unused=True,
    )
    _CACHE["runner"] = (sharded, in_names, zero_shapes)
    return _CACHE["runner"]


def kernel(**inputs):
    sharded, in_names, zero_shapes = _get_runner()

    x = np.ascontiguousarray(np.asarray(inputs["inputs"], dtype=np.float32))
    wq = np.ascontiguousarray(np.asarray(inputs["W_query"], dtype=np.float32))
    wk = np.ascontiguousarray(np.asarray(inputs["W_key"], dtype=np.float32))
    wv = np.ascontiguousarray(np.asarray(inputs["W_value"], dtype=np.float32))
    per_core = {
        "x": [x[b] for b in range(N_CORES)],
        "wq": [wq] * N_CORES,
        "wk": [wk] * N_CORES,
        "wv": [wv] * N_CORES,
    }
    concat_in = [np.concatenate(per_core[nm], axis=0) for nm in in_names]
    concat_zeros = [np.zeros((N_CORES * sh[0], *sh[1:]), dt)
                    for sh, dt in zero_shapes]
    outs = sharded(*concat_in, *concat_zeros)
    y = np.asarray(outs[0]).reshape(N_CORES, S, D)
    return y
